# revision 1
# baseline (speedup 1.0000x reference)
"""Neural BP decoder kernel for Trainium2 (8 NeuronCores).

Algorithm restructuring vs the reference:
  - iteration 0 of the reference acts on v2c = tile(llr) which is rank-1;
    its check/variable updates collapse to matvecs computed on the host.
  - every remaining iteration needs two dense 4096^3 matmuls:
      check:    R = H @ sign(v2c).T          (operands {0,+-1}: exact in fp16)
      variable: v2c' = llr + H.T @ c2v       (c2v = gamma*sign(R)*rowmag)
    The variable matmul runs as TWO fp16 matmuls with the magnitude split
    hi/lo (mag = hi + lo, each fp16), giving ~fp32-level end-to-end error
    (measured 7.7e-5 rel) at 2x the bf16 matmul rate cost.
  - sharding: core c owns rows B_c = [512c, 512c+512) of the check/variable
    index. It computes c2v rows B_c and v2c'.T columns B_c. The variable
    update is computed TRANSPOSED so the sign matrix lands in exactly the
    layout the next check matmul needs. Two AllGathers per iteration move
    the fp16 sign/c2v blocks between cores.
"""

import os
import numpy as np

import concourse.bass as bass
import concourse.mybir as mybir
import concourse.tile as tile
from concourse import bacc
from concourse.bass_utils import run_bass_kernel_spmd
from concourse.masks import make_identity

N = 4096
P = 128
NCORES = 8
BC = N // NCORES          # 512 rows per core
KT = N // P               # 32 k-tiles
MT = BC // P              # 4 m-tiles per core block
BIGF = 1.0e9

dt = mybir.dt
F32 = dt.float32
F16 = dt.float16
Alu = mybir.AluOpType
Act = mybir.ActivationFunctionType


def _build(n_steps: int, gamma: float):
    """Build the SPMD program for n_steps full device iterations."""
    nc = bacc.Bacc("TRN2", target_bir_lowering=False, debug=False)

    hct_d = nc.dram_tensor("hct", [N, BC], F16, kind="ExternalInput")
    hcol_d = nc.dram_tensor("hcol", [N, BC], F16, kind="ExternalInput")
    llrt_d = nc.dram_tensor("llrt", [P, KT], F32, kind="ExternalInput")
    ubc_d = nc.dram_tensor("ubc", [P, BC], F32, kind="ExternalInput")
    out_d = nc.dram_tensor("out_c", [N, BC], F32, kind="ExternalOutput")
    RG = [list(range(NCORES))]

    with tile.TileContext(nc) as tc:
        with tc.tile_pool(name="resid", bufs=1) as resid, \
             tc.tile_pool(name="slabp", bufs=2) as slabp, \
             tc.tile_pool(name="chunkp", bufs=4) as chunkp, \
             tc.tile_pool(name="work", bufs=2) as work, \
             tc.tile_pool(name="psp", bufs=8, space="PSUM") as psp, \
             tc.tile_pool(name="dram", bufs=2, space="DRAM") as dram:

            # ---- residents ----
            hct_sb = resid.tile([P, KT, BC], F16, tag="hct")
            hcol_sb = resid.tile([P, KT, BC], F16, tag="hcol")
            llrt_sb = resid.tile([P, KT], F32, tag="llrt")
            ubc_sb = resid.tile([P, BC], F32, tag="ubc")
            ident = resid.tile([P, P], F32, tag="ident")
            nc.sync.dma_start(hct_sb[:], hct_d.rearrange("(ko p) i -> p ko i", p=P))
            nc.sync.dma_start(hcol_sb[:], hcol_d.rearrange("(ko p) i -> p ko i", p=P))
            nc.sync.dma_start(llrt_sb[:], llrt_d[:])
            nc.sync.dma_start(ubc_sb[:], ubc_d[:])
            make_identity(nc, ident[:])

            def var_evac(jm, tt, stc, macc):
                """sign + masked |.| min accumulate for one v2cT row-tile."""
                st = work.tile([P, BC], F16, tag="st", name=f"st{jm}")
                nc.scalar.sign(st[:], tt[:])
                nc.sync.dma_start(stc[jm * P:(jm + 1) * P, :], st[:])
                aab = work.tile([P, BC], F32, tag="aab", name=f"aab{jm}")
                nc.scalar.activation(aab[:], tt[:], Act.Abs)
                hbig = work.tile([P, BC], F32, tag="hbig", name=f"hb{jm}")
                nc.vector.tensor_scalar(hbig[:], hct_sb[:, jm, :], -BIGF, BIGF,
                                        Alu.mult, Alu.add)
                msk = work.tile([P, BC], F32, tag="msk", name=f"mk{jm}")
                nc.vector.tensor_tensor(msk[:], aab[:], hbig[:], Alu.add)
                nc.vector.tensor_tensor(macc[:], macc[:], msk[:], Alu.min)

            def mag_chain(macc):
                """partition-min of macc -> per-row gamma*mag split hi/lo."""
                magt = work.tile([P, MT], F32, tag="magt")
                for cc in range(MT):
                    trp = psp.tile([P, P], F32, tag="ps", name=f"tr{cc}")
                    nc.tensor.transpose(trp[:], macc[:, cc * P:(cc + 1) * P], ident[:])
                    nc.vector.tensor_reduce(magt[:, cc:cc + 1], trp[:],
                                            axis=mybir.AxisListType.X, op=Alu.min)
                gm = work.tile([P, MT], F32, tag="gm")
                nc.vector.tensor_scalar(gm[:], magt[:], float(gamma), None, Alu.mult)
                gmhi16 = work.tile([P, MT], F16, tag="gmhi16")
                nc.vector.tensor_copy(gmhi16[:], gm[:])
                gmhi = work.tile([P, MT], F32, tag="gmhi")
                nc.vector.tensor_copy(gmhi[:], gmhi16[:])
                gmlo16 = work.tile([P, MT], F16, tag="gmlo16")
                nc.vector.tensor_tensor(gmlo16[:], gm[:], gmhi[:], Alu.subtract)
                gmlo = work.tile([P, MT], F32, tag="gmlo")
                nc.vector.tensor_copy(gmlo[:], gmlo16[:])
                return gmhi, gmlo

            # ---- prologue: v2cT_1[x, i] = llr[x] + u[i] (rank-1) ----
            stc = dram.tile([N, BC], F16, tag="stc", name="stc0")
            macc = work.tile([P, BC], F32, tag="macc", name="macc0")
            nc.vector.memset(macc[:], 3.0e38)
            for jm in range(KT):
                tt = work.tile([P, BC], F32, tag="tt", name=f"ptt{jm}")
                nc.vector.tensor_scalar(tt[:], ubc_sb[:], llrt_sb[:, jm:jm + 1],
                                        None, Alu.add)
                var_evac(jm, tt, stc, macc)
            gmhi, gmlo = mag_chain(macc)

            # ---- main iterations ----
            for t in range(1, n_steps + 1):
                last = (t == n_steps)

                gst = dram.tile([N * NCORES, BC], F16, tag="gst",
                                addr_space="Shared", name=f"gst{t}")
                nc.gpsimd.collective_compute(
                    "AllGather", Alu.bypass, replica_groups=RG,
                    ins=[stc.opt()], outs=[gst.opt()])

                # check: R_c = H_c @ S.T ; c2v = gamma*sign(R)*mag (hi/lo fp16)
                c2vl = dram.tile([2 * BC, N], F16, tag="c2vl", name=f"c2vl{t}")
                for nb in range(NCORES):
                    slab = slabp.tile([P, KT, BC], F16, tag="slab",
                                      name=f"sl{t}_{nb}")
                    nc.sync.dma_start(
                        slab[:],
                        gst[nb * N:(nb + 1) * N, :].rearrange(
                            "(ko p) j -> p ko j", p=P))
                    for m in range(MT):
                        ps = psp.tile([P, BC], F32, tag="ps",
                                      name=f"ck{t}_{nb}_{m}")
                        for k in range(KT):
                            nc.tensor.matmul(ps[:],
                                             hct_sb[:, k, m * P:(m + 1) * P],
                                             slab[:, k, :],
                                             start=(k == 0), stop=(k == KT - 1))
                        s = work.tile([P, BC], F32, tag="cks",
                                      name=f"cs{t}_{nb}_{m}")
                        nc.scalar.sign(s[:], ps[:])
                        hi = work.tile([P, BC], F16, tag="ckhi",
                                       name=f"ch{t}_{nb}_{m}")
                        nc.vector.tensor_scalar(hi[:], s[:], gmhi[:, m:m + 1],
                                                None, Alu.mult)
                        lo = work.tile([P, BC], F16, tag="cklo",
                                       name=f"cl{t}_{nb}_{m}")
                        nc.vector.tensor_scalar(lo[:], s[:], gmlo[:, m:m + 1],
                                                None, Alu.mult)
                        nc.sync.dma_start(
                            c2vl[m * P:(m + 1) * P, nb * BC:(nb + 1) * BC], hi[:])
                        nc.sync.dma_start(
                            c2vl[BC + m * P:BC + (m + 1) * P,
                                 nb * BC:(nb + 1) * BC], lo[:])

                gc2v = dram.tile([2 * BC * NCORES, N], F16, tag="gc2v",
                                 addr_space="Shared", name=f"gc2v{t}")
                nc.gpsimd.collective_compute(
                    "AllGather", Alu.bypass, replica_groups=RG,
                    ins=[c2vl.opt()], outs=[gc2v.opt()])

                # variable: v2cT' = llr + (c2v_hi + c2v_lo).T @ Hcol
                if not last:
                    stc = dram.tile([N, BC], F16, tag="stc", name=f"stc{t}")
                    macc = work.tile([P, BC], F32, tag="macc", name=f"macc{t}")
                    nc.vector.memset(macc[:], 3.0e38)
                for jg in range(8):
                    pss = [psp.tile([P, BC], F32, tag="ps",
                                    name=f"vp{t}_{jg}_{jj}") for jj in range(4)]
                    for kk in range(64):
                        d, h, it = kk // 8, (kk // 4) % 2, kk % 4
                        base = d * 2 * BC + h * BC + it * P
                        mt_idx = d * MT + it
                        chunk = chunkp.tile([P, BC], F16, tag="chunk",
                                            name=f"cku{t}_{jg}_{kk}")
                        nc.sync.dma_start(
                            chunk[:], gc2v[base:base + P, jg * BC:(jg + 1) * BC])
                        for jj in range(4):
                            nc.tensor.matmul(pss[jj][:],
                                             chunk[:, jj * P:(jj + 1) * P],
                                             hcol_sb[:, mt_idx, :],
                                             start=(kk == 0), stop=(kk == 63))
                    for jj in range(4):
                        jm = jg * 4 + jj
                        tt = work.tile([P, BC], F32, tag="tt",
                                       name=f"vt{t}_{jm}")
                        nc.vector.tensor_scalar(tt[:], pss[jj][:],
                                                llrt_sb[:, jm:jm + 1],
                                                None, Alu.add)
                        if last:
                            nc.sync.dma_start(out_d[jm * P:(jm + 1) * P, :], tt[:])
                        else:
                            var_evac(jm, tt, stc, macc)
                if not last:
                    gmhi, gmlo = mag_chain(macc)

    nc.compile()
    return nc


_PROGRAM_CACHE = {}


def _get_program(n_steps: int, gamma: float):
    key = (n_steps, float(gamma))
    if key not in _PROGRAM_CACHE:
        _PROGRAM_CACHE[key] = _build(n_steps, gamma)
    return _PROGRAM_CACHE[key]


def kernel(llr, H, gamma, n_iter, **kwargs):
    llr = np.asarray(llr, dtype=np.float32).reshape(N)
    H = np.ascontiguousarray(np.asarray(H, dtype=np.float32).reshape(N, N))
    gamma_f = float(np.asarray(gamma))
    n_iter_i = int(np.asarray(n_iter))
    assert n_iter_i >= 1

    # ---- host closed form for iteration 0 (v2c_0 = tile(llr) is rank-1) ----
    sllr = np.sign(llr).astype(np.float32)
    q = H @ sllr
    absllr = np.abs(llr).astype(np.float32)
    masked = np.where(H != 0, absllr[None, :], np.float32(BIGF))
    mag0 = np.min(masked, axis=1).astype(np.float32)
    c0 = (np.float32(gamma_f) * np.sign(q).astype(np.float32) * mag0).astype(np.float32)
    u = (H.T @ c0).astype(np.float32)

    if n_iter_i == 1:
        return (llr[None, :] + u[:, None]).astype(np.float32)

    n_steps = n_iter_i - 1
    nc = _get_program(n_steps, gamma_f)

    Hf16 = H.astype(np.float16)
    llrt = np.ascontiguousarray(llr.reshape(KT, P).T)        # [P, KT]
    in_maps = []
    for c in range(NCORES):
        sl = slice(c * BC, (c + 1) * BC)
        in_maps.append({
            "hct": np.ascontiguousarray(Hf16[sl, :].T),      # [N, BC]
            "hcol": np.ascontiguousarray(Hf16[:, sl]),       # [N, BC]
            "llrt": llrt,
            "ubc": np.ascontiguousarray(
                np.broadcast_to(u[sl], (P, BC))).astype(np.float32),
        })

    trace = bool(int(os.environ.get("NBP_TRACE", "0")))
    res = run_bass_kernel_spmd(nc, in_maps, core_ids=list(range(NCORES)),
                               trace=trace)
    if trace and res.exec_time_ns is not None:
        print(f"HW exec time: {res.exec_time_ns} ns")

    out = np.empty((N, N), dtype=np.float32)
    for c in range(NCORES):
        out[c * BC:(c + 1) * BC, :] = res.results[c]["out_c"].T
    return out


# revision 2
# speedup vs baseline: 1.3580x; 1.3580x over previous
"""Neural BP decoder kernel for Trainium2 (8 NeuronCores).

Algorithm restructuring vs the reference:
  - iteration 0 of the reference acts on v2c = tile(llr) which is rank-1;
    its check/variable updates collapse to matvecs computed on the host.
  - every remaining iteration needs two dense 4096^3 matmuls:
      check:    R = H @ sign(v2c).T          (operands {0,+-1}: exact in fp8)
      variable: v2c' = llr + H.T @ c2v       (c2v = gamma*sign(R)*rowmag)
    The variable matmul runs as TWO fp16 matmuls with the magnitude split
    hi/lo (mag = hi + lo, each fp16), giving ~fp32-level end-to-end error
    (measured ~8e-5 rel) at 2x the fp16 matmul rate cost.
  - sharding: core c owns rows B_c = [512c, 512c+512) of the check/variable
    index. It computes c2v rows B_c and v2c'.T columns B_c. The variable
    update is computed TRANSPOSED so the sign matrix lands in exactly the
    layout the next check matmul needs. AllGathers move the fp8 sign /
    fp16 c2v blocks between cores; both are split into quarters fired as
    soon as their producer slice completes, hiding them under compute.
"""

import os
import numpy as np

import concourse.bass as bass
import concourse.mybir as mybir
import concourse.tile as tile
from concourse import bacc
from concourse.bass_utils import run_bass_kernel_spmd
from concourse.masks import make_identity

N = 4096
P = 128
NCORES = 8
BC = N // NCORES          # 512 rows per core
KT = N // P               # 32 k-tiles
MT = BC // P              # 4 m-tiles per core block
BIGF = 1.0e9

dt = mybir.dt
F32 = dt.float32
F16 = dt.float16
F8 = dt.float8e4
Alu = mybir.AluOpType
Act = mybir.ActivationFunctionType


def _build(n_steps: int, gamma: float):
    """Build the SPMD program for n_steps full device iterations."""
    nc = bacc.Bacc("TRN2", target_bir_lowering=False, debug=False)

    hct_d = nc.dram_tensor("hct", [N, BC], F8, kind="ExternalInput")
    hcol_d = nc.dram_tensor("hcol", [N, BC], F16, kind="ExternalInput")
    llrt_d = nc.dram_tensor("llrt", [P, KT], F32, kind="ExternalInput")
    ubc_d = nc.dram_tensor("ubc", [P, BC], F32, kind="ExternalInput")
    out_d = nc.dram_tensor("out_c", [N, BC], F32, kind="ExternalOutput")
    RG = [list(range(NCORES))]

    with tile.TileContext(nc) as tc:
        with tc.tile_pool(name="resid", bufs=1) as resid, \
             tc.tile_pool(name="slabp", bufs=2) as slabp, \
             tc.tile_pool(name="chunkp", bufs=3) as chunkp, \
             tc.tile_pool(name="work", bufs=2) as work, \
             tc.tile_pool(name="psp", bufs=8, space="PSUM") as psp, \
             tc.tile_pool(name="dram", bufs=2, space="DRAM") as dram:

            # ---- residents ----
            hct_sb = resid.tile([P, KT, BC], F8, tag="hct")
            hcol_sb = resid.tile([P, KT, BC], F16, tag="hcol")
            llrt_sb = resid.tile([P, KT], F32, tag="llrt")
            ubc_sb = resid.tile([P, BC], F32, tag="ubc")
            ident = resid.tile([P, P], F32, tag="ident")
            nc.sync.dma_start(hct_sb[:], hct_d.rearrange("(ko p) i -> p ko i", p=P))
            nc.sync.dma_start(hcol_sb[:], hcol_d.rearrange("(ko p) i -> p ko i", p=P))
            nc.sync.dma_start(llrt_sb[:], llrt_d[:])
            nc.sync.dma_start(ubc_sb[:], ubc_d[:])
            make_identity(nc, ident[:])

            def ag(ins_ap, outs_ap):
                nc.gpsimd.collective_compute(
                    "AllGather", Alu.bypass, replica_groups=RG,
                    ins=[ins_ap], outs=[outs_ap])

            def var_evac(jm, tt, stc_q, macc):
                """sign + masked |.| min accumulate for one v2cT row-tile.

                stc_q: list of 4 row-quarter DRAM tiles [KT/4*P, BC] fp8."""
                st = work.tile([P, BC], F8, tag="st", name=f"st{jm}")
                nc.scalar.sign(st[:], tt[:])
                q, r = divmod(jm, KT // 4)
                nc.sync.dma_start(stc_q[q][r * P:(r + 1) * P, :], st[:])
                aab = work.tile([P, BC], F32, tag="aab", name=f"aab{jm}")
                nc.scalar.activation(aab[:], tt[:], Act.Abs)
                hbig = work.tile([P, BC], F32, tag="hbig", name=f"hb{jm}")
                nc.vector.tensor_scalar(hbig[:], hct_sb[:, jm, :], -BIGF, BIGF,
                                        Alu.mult, Alu.add)
                msk = work.tile([P, BC], F32, tag="msk", name=f"mk{jm}")
                nc.vector.tensor_tensor(msk[:], aab[:], hbig[:], Alu.add)
                nc.vector.tensor_tensor(macc[:], macc[:], msk[:], Alu.min)

            def mag_chain(macc):
                """partition-min of macc -> per-row gamma*mag split hi/lo."""
                magt = work.tile([P, MT], F32, tag="magt")
                for cc in range(MT):
                    trp = psp.tile([P, P], F32, tag="ps", name=f"tr{cc}")
                    nc.tensor.transpose(trp[:], macc[:, cc * P:(cc + 1) * P], ident[:])
                    nc.vector.tensor_reduce(magt[:, cc:cc + 1], trp[:],
                                            axis=mybir.AxisListType.X, op=Alu.min)
                gm = work.tile([P, MT], F32, tag="gm")
                nc.vector.tensor_scalar(gm[:], magt[:], float(gamma), None, Alu.mult)
                gmhi16 = work.tile([P, MT], F16, tag="gmhi16")
                nc.vector.tensor_copy(gmhi16[:], gm[:])
                gmhi = work.tile([P, MT], F32, tag="gmhi")
                nc.vector.tensor_copy(gmhi[:], gmhi16[:])
                gmlo16 = work.tile([P, MT], F16, tag="gmlo16")
                nc.vector.tensor_tensor(gmlo16[:], gm[:], gmhi[:], Alu.subtract)
                gmlo = work.tile([P, MT], F32, tag="gmlo")
                nc.vector.tensor_copy(gmlo[:], gmlo16[:])
                return gmhi, gmlo

            def new_stc(t):
                return [dram.tile([KT // 4 * P, BC], F8, tag=f"stc{q}",
                                  name=f"stc{t}_{q}") for q in range(4)]

            # ---- prologue: v2cT_1[x, i] = llr[x] + u[i] (rank-1) ----
            stc_q = new_stc(0)
            macc = work.tile([P, BC], F32, tag="macc", name="macc0")
            nc.vector.memset(macc[:], 3.0e38)
            for jm in range(KT):
                tt = work.tile([P, BC], F32, tag="tt", name=f"ptt{jm}")
                nc.vector.tensor_scalar(tt[:], ubc_sb[:], llrt_sb[:, jm:jm + 1],
                                        None, Alu.add)
                var_evac(jm, tt, stc_q, macc)
            gmhi, gmlo = mag_chain(macc)
            gst_q = [dram.tile([KT // 4 * P * NCORES, BC], F8, tag=f"gst{q}",
                               addr_space="Shared", name=f"gst0_{q}")
                     for q in range(4)]
            for q in range(4):
                ag(stc_q[q].opt(), gst_q[q].opt())

            # ---- main iterations ----
            QR = KT // 4 * P  # 1024 rows per stc quarter
            for t in range(1, n_steps + 1):
                last = (t == n_steps)

                # check: R_c = H_c @ S.T ; c2v = gamma*sign(R)*mag (hi/lo fp16)
                # c2v column-quarters: quarter cq covers output cols of
                # ranks nb = 2cq, 2cq+1.
                c2vq = [dram.tile([2 * BC, N // 4], F16, tag=f"c2v{cq}",
                                  name=f"c2v{t}_{cq}") for cq in range(4)]
                gc2vq = [dram.tile([2 * BC * NCORES, N // 4], F16,
                                   tag=f"gc2v{cq}", addr_space="Shared",
                                   name=f"gc2v{t}_{cq}") for cq in range(4)]
                for nb in range(NCORES):
                    slab = slabp.tile([P, KT, BC], F8, tag="slab",
                                      name=f"sl{t}_{nb}")
                    for q in range(4):
                        nc.sync.dma_start(
                            slab[:, q * (KT // 4):(q + 1) * (KT // 4), :],
                            gst_q[q][nb * QR:(nb + 1) * QR, :].rearrange(
                                "(ko p) j -> p ko j", p=P))
                    cq, col = divmod(nb, 2)
                    for m in range(MT):
                        ps = psp.tile([P, BC], F32, tag="ps",
                                      name=f"ck{t}_{nb}_{m}")
                        for k in range(KT):
                            nc.tensor.matmul(ps[:],
                                             hct_sb[:, k, m * P:(m + 1) * P],
                                             slab[:, k, :],
                                             start=(k == 0), stop=(k == KT - 1))
                        s = work.tile([P, BC], F32, tag="cks",
                                      name=f"cs{t}_{nb}_{m}")
                        nc.scalar.sign(s[:], ps[:])
                        hi = work.tile([P, BC], F16, tag="ckhi",
                                       name=f"ch{t}_{nb}_{m}")
                        nc.vector.tensor_scalar(hi[:], s[:], gmhi[:, m:m + 1],
                                                None, Alu.mult)
                        lo = work.tile([P, BC], F16, tag="cklo",
                                       name=f"cl{t}_{nb}_{m}")
                        nc.vector.tensor_scalar(lo[:], s[:], gmlo[:, m:m + 1],
                                                None, Alu.mult)
                        nc.sync.dma_start(
                            c2vq[cq][m * P:(m + 1) * P,
                                     col * BC:(col + 1) * BC], hi[:])
                        nc.sync.dma_start(
                            c2vq[cq][BC + m * P:BC + (m + 1) * P,
                                     col * BC:(col + 1) * BC], lo[:])
                    if col == 1:
                        ag(c2vq[cq].opt(), gc2vq[cq].opt())

                # variable: v2cT' = llr + (c2v_hi + c2v_lo).T @ Hcol
                if not last:
                    stc_q = new_stc(t)
                    macc = work.tile([P, BC], F32, tag="macc", name=f"macc{t}")
                    nc.vector.memset(macc[:], 3.0e38)
                    gst_q = [dram.tile([QR * NCORES, BC], F8, tag=f"gst{q}",
                                       addr_space="Shared", name=f"gst{t}_{q}")
                             for q in range(4)]
                for jg in range(8):
                    gsrc = gc2vq[jg // 2]
                    gcol = (jg % 2) * BC
                    pss = [psp.tile([P, BC], F32, tag="ps",
                                    name=f"vp{t}_{jg}_{jj}") for jj in range(4)]
                    for d in range(NCORES):
                        bigc = chunkp.tile([P, 8, BC], F16, tag="chunk",
                                           name=f"cku{t}_{jg}_{d}")
                        nc.sync.dma_start(
                            bigc[:],
                            gsrc[d * 2 * BC:(d + 1) * 2 * BC,
                                 gcol:gcol + BC].rearrange(
                                "(s p) j -> p s j", p=P))
                        for s8 in range(8):
                            mt_idx = d * MT + (s8 % MT)
                            for jj in range(4):
                                nc.tensor.matmul(
                                    pss[jj][:],
                                    bigc[:, s8, jj * P:(jj + 1) * P],
                                    hcol_sb[:, mt_idx, :],
                                    start=(d == 0 and s8 == 0),
                                    stop=(d == NCORES - 1 and s8 == 7))
                    for jj in range(4):
                        jm = jg * 4 + jj
                        tt = work.tile([P, BC], F32, tag="tt",
                                       name=f"vt{t}_{jm}")
                        nc.vector.tensor_scalar(tt[:], pss[jj][:],
                                                llrt_sb[:, jm:jm + 1],
                                                None, Alu.add)
                        if last:
                            nc.sync.dma_start(out_d[jm * P:(jm + 1) * P, :], tt[:])
                        else:
                            var_evac(jm, tt, stc_q, macc)
                    if not last and jg % 2 == 1:
                        q = jg // 2
                        ag(stc_q[q].opt(), gst_q[q].opt())
                if not last:
                    gmhi, gmlo = mag_chain(macc)

    nc.compile()
    return nc


_PROGRAM_CACHE = {}


def _get_program(n_steps: int, gamma: float):
    key = (n_steps, float(gamma))
    if key not in _PROGRAM_CACHE:
        _PROGRAM_CACHE[key] = _build(n_steps, gamma)
    return _PROGRAM_CACHE[key]


def kernel(llr, H, gamma, n_iter, **kwargs):
    import ml_dtypes

    llr = np.asarray(llr, dtype=np.float32).reshape(N)
    H = np.ascontiguousarray(np.asarray(H, dtype=np.float32).reshape(N, N))
    gamma_f = float(np.asarray(gamma))
    n_iter_i = int(np.asarray(n_iter))
    assert n_iter_i >= 1

    # ---- host closed form for iteration 0 (v2c_0 = tile(llr) is rank-1) ----
    sllr = np.sign(llr).astype(np.float32)
    q = H @ sllr
    absllr = np.abs(llr).astype(np.float32)
    masked = np.where(H != 0, absllr[None, :], np.float32(BIGF))
    mag0 = np.min(masked, axis=1).astype(np.float32)
    c0 = (np.float32(gamma_f) * np.sign(q).astype(np.float32) * mag0).astype(np.float32)
    u = (H.T @ c0).astype(np.float32)

    if n_iter_i == 1:
        return (llr[None, :] + u[:, None]).astype(np.float32)

    n_steps = n_iter_i - 1
    nc = _get_program(n_steps, gamma_f)

    Hf16 = H.astype(np.float16)
    Hf8 = H.astype(ml_dtypes.float8_e4m3)
    llrt = np.ascontiguousarray(llr.reshape(KT, P).T)        # [P, KT]
    in_maps = []
    for c in range(NCORES):
        sl = slice(c * BC, (c + 1) * BC)
        in_maps.append({
            "hct": np.ascontiguousarray(Hf8[sl, :].T),       # [N, BC] fp8
            "hcol": np.ascontiguousarray(Hf16[:, sl]),       # [N, BC] fp16
            "llrt": llrt,
            "ubc": np.ascontiguousarray(
                np.broadcast_to(u[sl], (P, BC))).astype(np.float32),
        })

    trace = bool(int(os.environ.get("NBP_TRACE", "0")))
    res = run_bass_kernel_spmd(nc, in_maps, core_ids=list(range(NCORES)),
                               trace=trace)
    if trace and res.exec_time_ns is not None:
        print(f"HW exec time: {res.exec_time_ns} ns")

    out = np.empty((N, N), dtype=np.float32)
    for c in range(NCORES):
        out[c * BC:(c + 1) * BC, :] = res.results[c]["out_c"].T
    return out
